# revision 30
# baseline (speedup 1.0000x reference)
"""DeepSeekMoE layer on 8 Trainium2 NeuronCores.

Strategy (expert-parallel, matching the sharding hint):
  - Host computes the (tiny) gate: softmax(x @ gate_w + gate_b), top-2
    routing, and per-expert token gather with capacity padding.  This is
    the control plane (<1% of FLOPs); all heavy matmuls run on device.
  - Each of the 8 cores owns E/8 = 4 routed experts: it receives the
    gathered tokens for those experts (transposed, bf16), the expert
    weights (bf16), and the per-(token,expert) combine weights.
  - The 2 shared experts are data-parallel over tokens: core c processes
    tokens [c*512, (c+1)*512), as two 256-token blocks that flow through
    the same pipeline as the routed blocks (with both shared experts
    accumulated in one K=2*FD matmul chain, and no combine weight).
  - Device per block: hT = w1^T @ xT (K=D in PSUM) -> Gelu(+b1) on ACT
    (hT stays FD-major) -> second matmul runs token-major: gT token-
    chunks are the stationary operand, w2 rows stream as the moving
    operand; the top-k combine weight is applied during PSUM eviction
    as a per-partition scalar multiply (tokens are partitions there)
    -> y[tok, D] staged row-wise in SBUF -> contiguous drains.
    Over-capacity experts (skewed routing) split into <=512-token
    sub-blocks that reuse the same pipeline.
  - All inputs are pre-swizzled on the host into the exact SBUF tile
    layouts so every DMA is a fully contiguous copy; DMA issue is split
    across both HWDGE rings (sync + scalar engines).
  - Host scatter-adds routed expert outputs + shared outputs back into
    token order (each token appears in exactly 2 routed lists + 1 shared
    list, so a fp32 sum reproduces the reference combine).
All matmul inputs are bf16 (PSUM accumulates fp32); biases are applied
in fp32 (b1 via the ACT bias port; b2/gate_b host-side, and they are
zero-guarded so the common all-zero case costs nothing).
"""

import os
import sys
import types

import numpy as np

# ---------------------------------------------------------------------------
# Optional NTFF trace support under axon: concourse's trace path imports
# antenv.axon_hooks, which this image lacks; shim it with the boot helper.
# ---------------------------------------------------------------------------
def _install_trace_shim():
    try:
        if "antenv.axon_hooks" in sys.modules:
            return
        from trn_agent_boot.trn_boot import _ntff_profile_via_ctypes

        hook = _ntff_profile_via_ctypes("/opt/axon/libaxon_pjrt.so")
        mod = types.ModuleType("antenv.axon_hooks")
        mod.get_axon_ntff_profile_hook = lambda: hook
        mod.set_axon_ntff_profile_hook = lambda h: None
        sys.modules["antenv.axon_hooks"] = mod
    except Exception:
        pass


_install_trace_shim()

import ml_dtypes  # noqa: E402

import concourse.bacc as bacc  # noqa: E402
import concourse.mybir as mybir  # noqa: E402
import concourse.tile as tile  # noqa: E402
import concourse.bass_utils as bass_utils  # noqa: E402
from concourse.bass import ts, ds  # noqa: E402
from concourse.bass_utils import run_bass_kernel_spmd  # noqa: E402

try:  # zero-egress sandbox: skip artifact uploads during tracing
    bass_utils.upload_artifacts = lambda tmpdir: tmpdir
except Exception:
    pass

# Problem shapes (nn_DeepSeekMoE): x [B,T,D]; E routed experts (top-K),
# S shared experts, ffn dim FD.
B, T, D = 2, 2048, 5120
FD, E, S, K = 384, 32, 2, 2
N = B * T                     # 4096 tokens
P = 128
NCORES = 8
EPC = E // NCORES             # 4 routed experts per core
NS = N // NCORES              # 512 shared-expert tokens per core
NSH = NS // 2                 # shared-block token count (256)
KD = D // P                   # 40 K-subtiles over D
KF = FD // P                  # 3 K-subtiles over FD
NDC = D // 512                # 10 output-column chunks of 512 over D

BF16 = ml_dtypes.bfloat16

LAST_EXEC_NS = None
LAST_MEAN_EXEC_NS = None
LAST_TRACE = None


def _ceil_div(a, b):
    return -(-a // b)


def _build_nc(C: int, NB: int = 1, out_dt=mybir.dt.bfloat16):
    """Build the SPMD per-core Bass program. C = per-expert token capacity.

    Input DRAM layouts are pre-swizzled on the host so that every DMA below
    is a contiguous copy:
      xep  [EPC, P, KD, C]        x tokens for expert j, transposed, p-major
      w1p  [EPC, P, KD, FD]       routed w1, p-major over D
      w2p  [EPC, NDC, P, KF, 512] routed w2, p-major over FD, chunked over D
      xsp  [2, P, KD, NSH]        shared tokens, 2 blocks, transposed, p-major
      sw1p [S, P, KD, FD]         shared w1
      sw2p [S, NDC, P, KF, 512]   shared w2
      wgt  [EPC*NB, ceil(C/P)*P]  combine weights, token-chunk-major
    Outputs are token-major: yE [EPC, C, D], ysE [NS, D].
    """
    f32 = mybir.dt.float32
    bf16 = mybir.dt.bfloat16
    nc = bacc.Bacc(None, target_bir_lowering=False)

    NEB = EPC * NB             # routed sub-blocks (<=512 tokens each)
    xep = nc.dram_tensor("xep", (NEB, P, KD, C), bf16, kind="ExternalInput")
    CP = _ceil_div(C, P) * P
    wgt = nc.dram_tensor("wgt", (NEB, CP), f32, kind="ExternalInput")
    w1p = nc.dram_tensor("w1p", (EPC, P, KD, FD), bf16, kind="ExternalInput")
    rb1 = nc.dram_tensor("rb1", (EPC, FD), f32, kind="ExternalInput")
    w2p = nc.dram_tensor("w2p", (EPC, NDC // 2, P, KF, 1024), bf16, kind="ExternalInput")
    xsp = nc.dram_tensor("xsp", (2, P, KD, NSH), bf16, kind="ExternalInput")
    sw1p = nc.dram_tensor("sw1p", (S, P, KD, FD), bf16, kind="ExternalInput")
    sb1 = nc.dram_tensor("sb1", (S, FD), f32, kind="ExternalInput")
    sw2p = nc.dram_tensor("sw2p", (S, NDC // 2, P, KF, 1024), bf16, kind="ExternalInput")
    yE = nc.dram_tensor("yE", (NEB, C, D), out_dt, kind="ExternalOutput")
    ysE = nc.dram_tensor("ysE", (NS, D), out_dt, kind="ExternalOutput")

    gelu = mybir.ActivationFunctionType.Gelu
    KDH = KD // 2              # w1 / x stream in two K-halves

    with tile.TileContext(nc) as tc:
        with (
            tc.tile_pool(name="pw1", bufs=4) as pw1,
            tc.tile_pool(name="pxe", bufs=4) as pxe,
            tc.tile_pool(name="pw2", bufs=4) as pw2,
            tc.tile_pool(name="pg", bufs=6) as pg,
            tc.tile_pool(name="pyt", bufs=4) as pyt,
            tc.tile_pool(name="pb", bufs=1) as pb,
            tc.tile_pool(name="pps1", bufs=2, space="PSUM") as pps1,
            tc.tile_pool(name="pps2", bufs=6, space="PSUM") as pps2,
        ):
            # biases -> [P, n_subtiles] with the FD subtile index on free dim
            # (emitted after the first expert's input DMAs so they don't
            # delay the first matmul in the queue)
            rb1_sb = pb.tile([P, EPC * KF], f32, tag="rb1")
            sb1_sb = pb.tile([P, S * KF], f32, tag="sb1")

            def emit_biases():
                for e in range(EPC):
                    nc.sync.dma_start(
                        rb1_sb[:, e * KF : (e + 1) * KF],
                        rb1[:][e].rearrange("(o p) -> p o", p=P),
                    )
                for s in range(S):
                    nc.sync.dma_start(
                        sb1_sb[:, s * KF : (s + 1) * KF],
                        sb1[:][s].rearrange("(o p) -> p o", p=P),
                    )

            def load_khalves(pool, src_ap, width, tag, eng, npieces=1):
                """Two [P, KD/2, width] tiles for a [P, KD, width] DRAM src."""
                tiles = []
                for h in range(2):
                    t = pool.tile([P, KDH, width], bf16, tag=tag, name="kh")
                    src = src_ap[:, h * KDH : (h + 1) * KDH]
                    step = KDH // npieces
                    for i in range(npieces):
                        eng.dma_start(
                            t[:, i * step : (i + 1) * step],
                            src[:, i * step : (i + 1) * step],
                        )
                    tiles.append(t)
                return tiles

            def mm1(x_tiles, ntok, w1_t, bias_sb, boff):
                """[P, KF, ntok] bf16 tile of gelu(w1^T x + b1)."""
                g_t = pg.tile([P, KF, ntok], bf16, tag="g", name="g_t")
                for mi in range(KF):
                    ph = pps1.tile([P, 512], f32, tag="ph", name="ph")[:, :ntok]
                    for kd in range(KD):
                        nc.tensor.matmul(
                            ph,
                            w1_t[kd // KDH][:, kd % KDH, ts(mi, P)],
                            x_tiles[kd // KDH][:, kd % KDH, :],
                            start=(kd == 0),
                            stop=(kd == KD - 1),
                        )
                    nc.scalar.activation(
                        g_t[:, mi, :],
                        ph,
                        gelu,
                        bias=bias_sb[:, boff + mi : boff + mi + 1],
                    )
                return g_t

            def mm2(gblocks, w2_list, ntoks, out_aps, scales):
                """Token-major second matmul over one or more token blocks.

                gblocks: per block, per source: [P, KF, ntok] bf16 g tiles
                w2_list: per source, [NDC, P, KF, 512] DRAM AP (streamed once)
                out_aps: per block, [ntok, D] DRAM AP
                scales:  per block, None or fn(ci, cw) -> [cw, 1] combine-
                         weight AP applied during PSUM eviction (tokens are
                         the PSUM partitions here, so a per-partition
                         tensor_scalar multiply applies the top-k weight)
                """
                nsrc = len(w2_list)
                nmm = nsrc * KF
                drain_after = {3: (0, 2048), 7: (2048, 4096)}
                drain_final = (4096, D)
                yrows = [
                    [
                        pyt.tile([P, D], out_dt, tag="yrow", name="yrow")
                        for _ in range(_ceil_div(ntok, P))
                    ]
                    for ntok in ntoks
                ]
                w2pair = [None] * nsrc
                for mdc in range(NDC):
                    if mdc % 2 == 0:
                        for si in range(nsrc):
                            w2t = pw2.tile(
                                [P, KF, 1024], bf16, tag="w2", name="w2t"
                            )
                            nc.scalar.dma_start(w2t, w2_list[si][mdc // 2])
                            w2pair[si] = w2t
                    off = (mdc % 2) * 512
                    w2ts = [w2pair[si][:, :, off : off + 512] for si in range(nsrc)]
                    for bi, gsrcs in enumerate(gblocks):
                        for ci in range(_ceil_div(ntoks[bi], P)):
                            cw = min(P, ntoks[bi] - ci * P)
                            py = pps2.tile(
                                [P, 512], f32, tag="py", name="py"
                            )[:cw]
                            imm = 0
                            for si in range(nsrc):
                                for kf in range(KF):
                                    nc.tensor.matmul(
                                        py,
                                        gsrcs[si][:, kf, ds(ci * P, cw)],
                                        w2ts[si][:, kf, :],
                                        start=(imm == 0),
                                        stop=(imm == nmm - 1),
                                    )
                                    imm += 1
                            dst = yrows[bi][ci][:cw, ts(mdc, 512)]
                            use_act = (mdc + ci) % 3 == 2
                            if scales[bi] is not None:
                                if use_act:
                                    nc.scalar.activation(
                                        dst, py,
                                        mybir.ActivationFunctionType.Copy,
                                        scale=scales[bi](ci, cw),
                                    )
                                else:
                                    nc.vector.tensor_scalar_mul(
                                        dst, py, scales[bi](ci, cw)
                                    )
                            elif use_act:
                                nc.scalar.activation(
                                    dst, py, mybir.ActivationFunctionType.Copy
                                )
                            else:
                                nc.vector.tensor_copy(out=dst, in_=py)
                    if mdc in drain_after:
                        lo, hi = drain_after[mdc]
                        for bi, ntok in enumerate(ntoks):
                            for ci in range(_ceil_div(ntok, P)):
                                cw = min(P, ntok - ci * P)
                                nc.gpsimd.dma_start(
                                    out_aps[bi][ds(ci * P, cw), lo:hi],
                                    yrows[bi][ci][:cw, lo:hi],
                                )
                lo, hi = drain_final
                for bi, ntok in enumerate(ntoks):
                    for ci in range(_ceil_div(ntok, P)):
                        cw = min(P, ntok - ci * P)
                        eng = nc.sync if ci % 2 == 0 else nc.scalar
                        eng.dma_start(
                            out_aps[bi][ds(ci * P, cw), lo:hi],
                            yrows[bi][ci][:cw, lo:hi],
                        )

            def load_first_block_interleaved(npieces=4):
                """Interleave the first shared block's sw1(s0)/xs(h0) piece
                DMAs across both HWDGE rings so the very first matmul's
                inputs are at the head of the queues."""
                w1_t, x_tiles = [], []
                for h in range(2):
                    w1h = pw1.tile([P, KDH, FD], bf16, tag="w1", name="w1h")
                    xh = pxe.tile([P, KDH, NSH], bf16, tag="xe", name="xh")
                    step = KDH // npieces
                    for i in range(npieces):
                        sl = slice(i * step, (i + 1) * step)
                        nc.sync.dma_start(
                            w1h[:, sl], sw1p[:][0][:, h * KDH :][:, sl]
                        )
                        nc.scalar.dma_start(
                            xh[:, sl], xsp[:][0][:, h * KDH :][:, sl]
                        )
                    w1_t.append(w1h)
                    x_tiles.append(xh)
                return w1_t, x_tiles

            # combine weights, token-chunk-major: [P, NCH] per sub-block
            NCH = _ceil_div(C, P)
            wgt_sb = pb.tile([P, NEB * NCH], f32, tag="wg")

            def routed_scale(eb):
                return lambda ci, cw: wgt_sb[:cw, eb * NCH + ci : eb * NCH + ci + 1]

            # ---------------- shared experts first (token-parallel) --------
            # The shared phase is PE-bound and DMA-light, so running it first
            # lets the DMA-bound routed phase prefetch its inputs behind the
            # shared matmuls.  h-outer / s-inner ordering; one fused mm2
            # streams each shared w2 once for all four (h, s) sub-blocks.
            sw1_t = [None, None]
            sw1_t[0], xs_h0 = load_first_block_interleaved()
            sw1_t[1] = load_khalves(pw1, sw1p[:][1], FD, "w1", nc.scalar)
            emit_biases()
            xs_tiles = [xs_h0, load_khalves(pxe, xsp[:][1], NSH, "xe", nc.sync)]
            gblocks = [[None] * S for _ in range(2)]
            for s in range(S):
                for h in range(2):
                    gblocks[h][s] = mm1(xs_tiles[h], NSH, sw1_t[s], sb1_sb, s * KF)
            # shared mm2 is deferred to the very end: its w2 stream (7.9MB)
            # would otherwise collide with the routed input streams mid-kernel,
            # while at the end the PE has ~70us of matmuls and the DMA rings
            # are otherwise idle.
            shared_pending = (
                gblocks,
                [sw2p[:][s] for s in range(S)],
                [NSH, NSH],
                [ysE[:][ds(h * NSH, NSH), :] for h in range(2)],
                [None, None],
            )
            pending = None

            # ---------------- routed experts ----------------
            # software pipeline: mm1(block i+1) is emitted before mm2(block i)
            # so the PE never waits on the gelu tail or w2 prefetch at a
            # block boundary.
            for eb in range(NEB):
                e = eb // NB
                w1_t = load_khalves(pw1, w1p[:][e], FD, "w1", nc.sync)
                x_tiles = load_khalves(pxe, xep[:][eb], C, "xe", nc.sync)
                nc.sync.dma_start(
                    wgt_sb[:, eb * NCH : (eb + 1) * NCH],
                    wgt[:][eb].rearrange("(o p) -> p o", p=P),
                )
                gt = mm1(x_tiles, C, w1_t, rb1_sb, e * KF)
                if pending is not None:
                    mm2(*pending)
                pending = ([[gt]], [w2p[:][e]], [C], [yE[:][eb]],
                           [routed_scale(eb)])
            mm2(*pending)
            mm2(*shared_pending)

    nc.compile()
    return nc


def kernel(x, shared_w1, shared_b1, shared_w2, shared_b2,
           routed_w1, routed_b1, routed_w2, routed_b2, gate_w, gate_b):
    global LAST_EXEC_NS, LAST_MEAN_EXEC_NS, LAST_TRACE

    x = np.asarray(x, np.float32)
    x2d = np.ascontiguousarray(x.reshape(N, D))

    # ---- gating / routing (control plane) ----
    logits = x2d @ np.asarray(gate_w, np.float32) + np.asarray(gate_b, np.float32)
    logits -= logits.max(axis=-1, keepdims=True)
    probs = np.exp(logits)
    probs /= probs.sum(axis=-1, keepdims=True)                  # [N, E]
    top2 = np.argpartition(-probs, K - 1, axis=-1)[:, :K]       # [N, K]
    sel = np.zeros((N, E), np.bool_)
    sel[np.arange(N)[:, None], top2] = True

    idx_per_e = [np.nonzero(sel[:, e])[0] for e in range(E)]
    counts = np.array([len(i) for i in idx_per_e])
    cmax = max(128, int(np.ceil(counts.max() / 32)) * 32)
    NB = _ceil_div(cmax, 512)        # sub-blocks per expert (1 unless skewed)
    C = max(128, int(np.ceil(cmax / NB / 32)) * 32)

    x_bf = x2d.astype(BF16)

    def pmajor_T(rows):
        """[n, D] fp32/bf16 rows -> x^T p-major [P, KD, n]."""
        return rows.T.reshape(KD, P, rows.shape[0]).transpose(1, 0, 2)

    # pre-swizzled shared weights (identical on every core)
    sw1p = np.ascontiguousarray(
        np.asarray(shared_w1).astype(BF16).reshape(S, KD, P, FD).transpose(0, 2, 1, 3)
    )
    sw2p = np.ascontiguousarray(
        np.asarray(shared_w2).astype(BF16)
        .reshape(S, KF, P, NDC // 2, 1024).transpose(0, 3, 2, 1, 4)
    )
    rw1_bf = np.asarray(routed_w1).astype(BF16)
    rw2_bf = np.asarray(routed_w2).astype(BF16)
    rb1_f = np.asarray(routed_b1, np.float32)
    sb1_f = np.asarray(shared_b1, np.float32)

    CP = _ceil_div(C, P) * P
    in_maps = []
    for c in range(NCORES):
        xep = np.zeros((EPC * NB, P, KD, C), BF16)
        wgtb = np.zeros((EPC * NB, CP), np.float32)
        for j in range(EPC):
            e = c * EPC + j
            for b in range(NB):
                idx = idx_per_e[e][b * C : (b + 1) * C]
                if len(idx) == 0:
                    continue
                xep[j * NB + b, :, :, : len(idx)] = pmajor_T(x_bf[idx])
                wgtb[j * NB + b, : len(idx)] = probs[idx, e]
        xsp = np.stack(
            [
                pmajor_T(x_bf[c * NS + h * NSH : c * NS + (h + 1) * NSH])
                for h in range(2)
            ]
        )
        w1c = rw1_bf[c * EPC : (c + 1) * EPC]
        w2c = rw2_bf[c * EPC : (c + 1) * EPC]
        in_maps.append(
            {
                "xep": xep,
                "wgt": wgtb,
                "w1p": np.ascontiguousarray(
                    w1c.reshape(EPC, KD, P, FD).transpose(0, 2, 1, 3)
                ),
                "rb1": np.ascontiguousarray(rb1_f[c * EPC : (c + 1) * EPC]),
                "w2p": np.ascontiguousarray(
                    w2c.reshape(EPC, KF, P, NDC // 2, 1024).transpose(0, 3, 2, 1, 4)
                ),
                "xsp": np.ascontiguousarray(xsp),
                "sw1p": sw1p,
                "sb1": sb1_f,
                "sw2p": sw2p,
            }
        )

    nc = _build_nc(C, NB)
    trace = bool(int(os.environ.get("MOE_TRACE", "0")))
    res = None
    for attempt in range(3):
        try:
            res = run_bass_kernel_spmd(
                nc, in_maps, core_ids=list(range(NCORES)), trace=trace
            )
            break
        except Exception:
            if attempt == 2:
                raise
    LAST_EXEC_NS = res.exec_time_ns
    LAST_MEAN_EXEC_NS = res.mean_exec_time_ns
    LAST_TRACE = res.instructions_and_trace[1] if res.instructions_and_trace else None

    # ---- combine (un-shard) ----
    out = np.zeros((N, D), np.float32)
    for c in range(NCORES):
        out[c * NS : (c + 1) * NS] = np.asarray(res.results[c]["ysE"], np.float32)
    for c in range(NCORES):
        yEc = res.results[c]["yE"]
        for j in range(EPC):
            e = c * EPC + j
            for b in range(NB):
                idx = idx_per_e[e][b * C : (b + 1) * C]
                if len(idx):
                    out[idx] += np.asarray(
                        yEc[j * NB + b, : len(idx), :], np.float32
                    )

    # host-side bias terms (zero in the shipped init; guarded for generality)
    shared_b2 = np.asarray(shared_b2, np.float32)
    if np.any(shared_b2):
        out += shared_b2.sum(0)[None, :]
    routed_b2 = np.asarray(routed_b2, np.float32)
    if np.any(routed_b2):
        w_full = np.where(sel, probs, 0.0).astype(np.float32)
        out += w_full @ routed_b2

    return out.reshape(B, T, D)


# revision 31
# speedup vs baseline: 1.0456x; 1.0456x over previous
"""DeepSeekMoE layer on 8 Trainium2 NeuronCores.

Strategy (expert-parallel, matching the sharding hint):
  - Host computes the (tiny) gate: softmax(x @ gate_w + gate_b), top-2
    routing, and per-expert token gather with capacity padding.  This is
    the control plane (<1% of FLOPs); all heavy matmuls run on device.
  - Each of the 8 cores owns E/8 = 4 routed experts: it receives the
    gathered tokens for those experts (transposed, bf16), the expert
    weights (bf16), and the per-(token,expert) combine weights.
  - The 2 shared experts are data-parallel over tokens: core c processes
    tokens [c*512, (c+1)*512), as two 256-token blocks that flow through
    the same pipeline as the routed blocks (with both shared experts
    accumulated in one K=2*FD matmul chain, and no combine weight).
  - Device per block: hT = w1^T @ xT (K=D in PSUM) -> Gelu(+b1) on ACT
    (hT stays FD-major) -> second matmul runs token-major: gT token-
    chunks are the stationary operand, w2 rows stream as the moving
    operand; the top-k combine weight is applied during PSUM eviction
    as a per-partition scalar multiply (tokens are partitions there)
    -> y[tok, D] staged row-wise in SBUF -> contiguous drains.
    Over-capacity experts (skewed routing) split into <=512-token
    sub-blocks that reuse the same pipeline.
  - All inputs are pre-swizzled on the host into the exact SBUF tile
    layouts so every DMA is a fully contiguous copy; DMA issue is split
    across both HWDGE rings (sync + scalar engines).
  - Host scatter-adds routed expert outputs + shared outputs back into
    token order (each token appears in exactly 2 routed lists + 1 shared
    list, so a fp32 sum reproduces the reference combine).
All matmul inputs are bf16 (PSUM accumulates fp32); biases are applied
in fp32 (b1 via the ACT bias port; b2/gate_b host-side, and they are
zero-guarded so the common all-zero case costs nothing).
"""

import os
import sys
import types

import numpy as np

# ---------------------------------------------------------------------------
# Optional NTFF trace support under axon: concourse's trace path imports
# antenv.axon_hooks, which this image lacks; shim it with the boot helper.
# ---------------------------------------------------------------------------
def _install_trace_shim():
    try:
        if "antenv.axon_hooks" in sys.modules:
            return
        from trn_agent_boot.trn_boot import _ntff_profile_via_ctypes

        hook = _ntff_profile_via_ctypes("/opt/axon/libaxon_pjrt.so")
        mod = types.ModuleType("antenv.axon_hooks")
        mod.get_axon_ntff_profile_hook = lambda: hook
        mod.set_axon_ntff_profile_hook = lambda h: None
        sys.modules["antenv.axon_hooks"] = mod
    except Exception:
        pass


_install_trace_shim()

import ml_dtypes  # noqa: E402

import concourse.bacc as bacc  # noqa: E402
import concourse.mybir as mybir  # noqa: E402
import concourse.tile as tile  # noqa: E402
import concourse.bass_utils as bass_utils  # noqa: E402
from concourse.bass import ts, ds  # noqa: E402
from concourse.bass_utils import run_bass_kernel_spmd  # noqa: E402

try:  # zero-egress sandbox: skip artifact uploads during tracing
    bass_utils.upload_artifacts = lambda tmpdir: tmpdir
except Exception:
    pass

# Problem shapes (nn_DeepSeekMoE): x [B,T,D]; E routed experts (top-K),
# S shared experts, ffn dim FD.
B, T, D = 2, 2048, 5120
FD, E, S, K = 384, 32, 2, 2
N = B * T                     # 4096 tokens
P = 128
NCORES = 8
EPC = E // NCORES             # 4 routed experts per core
NS = N // NCORES              # 512 shared-expert tokens per core
NSH = NS // 2                 # shared-block token count (256)
KD = D // P                   # 40 K-subtiles over D
KF = FD // P                  # 3 K-subtiles over FD
NDC = D // 512                # 10 output-column chunks of 512 over D

BF16 = ml_dtypes.bfloat16

LAST_EXEC_NS = None
LAST_MEAN_EXEC_NS = None
LAST_TRACE = None


def _ceil_div(a, b):
    return -(-a // b)


def _build_nc(C: int, NB: int = 1, out_dt=mybir.dt.bfloat16):
    """Build the SPMD per-core Bass program. C = per-expert token capacity.

    Input DRAM layouts are pre-swizzled on the host so that every DMA below
    is a contiguous copy:
      xep  [EPC, P, KD, C]        x tokens for expert j, transposed, p-major
      w1p  [EPC, P, KD, FD]       routed w1, p-major over D
      w2p  [EPC, NDC, P, KF, 512] routed w2, p-major over FD, chunked over D
      xsp  [2, P, KD, NSH]        shared tokens, 2 blocks, transposed, p-major
      sw1p [S, P, KD, FD]         shared w1
      sw2p [S, NDC, P, KF, 512]   shared w2
      wgt  [EPC*NB, ceil(C/P)*P]  combine weights, token-chunk-major
    Outputs are token-major: yE [EPC, C, D], ysE [NS, D].
    """
    f32 = mybir.dt.float32
    bf16 = mybir.dt.bfloat16
    nc = bacc.Bacc(None, target_bir_lowering=False)

    NEB = EPC * NB             # routed sub-blocks (<=512 tokens each)
    xep = nc.dram_tensor("xep", (NEB, P, KD, C), bf16, kind="ExternalInput")
    CP = _ceil_div(C, P) * P
    wgt = nc.dram_tensor("wgt", (NEB, CP), f32, kind="ExternalInput")
    w1p = nc.dram_tensor("w1p", (EPC, P, KD, FD), bf16, kind="ExternalInput")
    rb1 = nc.dram_tensor("rb1", (EPC, FD), f32, kind="ExternalInput")
    w2p = nc.dram_tensor("w2p", (EPC, NDC // 2, P, KF, 1024), bf16, kind="ExternalInput")
    xsp = nc.dram_tensor("xsp", (2, P, KD, NSH), bf16, kind="ExternalInput")
    sw1p = nc.dram_tensor("sw1p", (S, P, KD, FD), bf16, kind="ExternalInput")
    sb1 = nc.dram_tensor("sb1", (S, FD), f32, kind="ExternalInput")
    sw2p = nc.dram_tensor("sw2p", (S, NDC // 2, P, KF, 1024), bf16, kind="ExternalInput")
    yE = nc.dram_tensor("yE", (NEB, C, D), out_dt, kind="ExternalOutput")
    ysE = nc.dram_tensor("ysE", (NS, D), out_dt, kind="ExternalOutput")

    gelu = mybir.ActivationFunctionType.Gelu
    KDH = KD // 2              # w1 / x stream in two K-halves

    with tile.TileContext(nc) as tc:
        with (
            tc.tile_pool(name="pw1", bufs=4) as pw1,
            tc.tile_pool(name="pxe", bufs=4) as pxe,
            tc.tile_pool(name="pw2", bufs=4) as pw2,
            tc.tile_pool(name="pg", bufs=6) as pg,
            tc.tile_pool(name="pyt", bufs=4) as pyt,
            tc.tile_pool(name="pb", bufs=1) as pb,
            tc.tile_pool(name="pps1", bufs=2, space="PSUM") as pps1,
            tc.tile_pool(name="pps2", bufs=6, space="PSUM") as pps2,
        ):
            # biases -> [P, n_subtiles] with the FD subtile index on free dim
            # (emitted after the first expert's input DMAs so they don't
            # delay the first matmul in the queue)
            rb1_sb = pb.tile([P, EPC * KF], f32, tag="rb1")
            sb1_sb = pb.tile([P, S * KF], f32, tag="sb1")

            def emit_biases():
                for e in range(EPC):
                    nc.sync.dma_start(
                        rb1_sb[:, e * KF : (e + 1) * KF],
                        rb1[:][e].rearrange("(o p) -> p o", p=P),
                    )
                for s in range(S):
                    nc.sync.dma_start(
                        sb1_sb[:, s * KF : (s + 1) * KF],
                        sb1[:][s].rearrange("(o p) -> p o", p=P),
                    )

            def load_khalves(pool, src_ap, width, tag, eng, npieces=1):
                """Two [P, KD/2, width] tiles for a [P, KD, width] DRAM src."""
                tiles = []
                for h in range(2):
                    t = pool.tile([P, KDH, width], bf16, tag=tag, name="kh")
                    src = src_ap[:, h * KDH : (h + 1) * KDH]
                    step = KDH // npieces
                    for i in range(npieces):
                        eng.dma_start(
                            t[:, i * step : (i + 1) * step],
                            src[:, i * step : (i + 1) * step],
                        )
                    tiles.append(t)
                return tiles

            def mm1(x_tiles, ntok, w1_t, bias_sb, boff):
                """[P, KF, ntok] bf16 tile of gelu(w1^T x + b1)."""
                g_t = pg.tile([P, KF, ntok], bf16, tag="g", name="g_t")
                for mi in range(KF):
                    ph = pps1.tile([P, 512], f32, tag="ph", name="ph")[:, :ntok]
                    for kd in range(KD):
                        nc.tensor.matmul(
                            ph,
                            w1_t[kd // KDH][:, kd % KDH, ts(mi, P)],
                            x_tiles[kd // KDH][:, kd % KDH, :],
                            start=(kd == 0),
                            stop=(kd == KD - 1),
                        )
                    nc.scalar.activation(
                        g_t[:, mi, :],
                        ph,
                        gelu,
                        bias=bias_sb[:, boff + mi : boff + mi + 1],
                    )
                return g_t

            def mm2(gblocks, w2_list, ntoks, out_aps, scales):
                """Token-major second matmul over one or more token blocks.

                gblocks: per block, per source: [P, KF, ntok] bf16 g tiles
                w2_list: per source, [NDC, P, KF, 512] DRAM AP (streamed once)
                out_aps: per block, [ntok, D] DRAM AP
                scales:  per block, None or fn(ci, cw) -> [cw, 1] combine-
                         weight AP applied during PSUM eviction (tokens are
                         the PSUM partitions here, so a per-partition
                         tensor_scalar multiply applies the top-k weight)
                """
                nsrc = len(w2_list)
                nmm = nsrc * KF
                drain_after = {3: (0, 2048), 7: (2048, 4096)}
                drain_final = (4096, D)
                yrows = [
                    [
                        pyt.tile([P, D], out_dt, tag="yrow", name="yrow")
                        for _ in range(_ceil_div(ntok, P))
                    ]
                    for ntok in ntoks
                ]
                w2pair = [None] * nsrc
                for mdc in range(NDC):
                    if mdc % 2 == 0:
                        for si in range(nsrc):
                            w2t = pw2.tile(
                                [P, KF, 1024], bf16, tag="w2", name="w2t"
                            )
                            nc.scalar.dma_start(w2t, w2_list[si][mdc // 2])
                            w2pair[si] = w2t
                    off = (mdc % 2) * 512
                    w2ts = [w2pair[si][:, :, off : off + 512] for si in range(nsrc)]
                    for bi, gsrcs in enumerate(gblocks):
                        for ci in range(_ceil_div(ntoks[bi], P)):
                            cw = min(P, ntoks[bi] - ci * P)
                            py = pps2.tile(
                                [P, 512], f32, tag="py", name="py"
                            )[:cw]
                            imm = 0
                            for si in range(nsrc):
                                for kf in range(KF):
                                    nc.tensor.matmul(
                                        py,
                                        gsrcs[si][:, kf, ds(ci * P, cw)],
                                        w2ts[si][:, kf, :],
                                        start=(imm == 0),
                                        stop=(imm == nmm - 1),
                                    )
                                    imm += 1
                            dst = yrows[bi][ci][:cw, ts(mdc, 512)]
                            use_act = (mdc + ci) % 3 == 2
                            if scales[bi] is not None:
                                if use_act:
                                    nc.scalar.activation(
                                        dst, py,
                                        mybir.ActivationFunctionType.Copy,
                                        scale=scales[bi](ci, cw),
                                    )
                                else:
                                    nc.vector.tensor_scalar_mul(
                                        dst, py, scales[bi](ci, cw)
                                    )
                            elif use_act:
                                nc.scalar.activation(
                                    dst, py, mybir.ActivationFunctionType.Copy
                                )
                            else:
                                nc.vector.tensor_copy(out=dst, in_=py)
                    if mdc in drain_after:
                        lo, hi = drain_after[mdc]
                        for bi, ntok in enumerate(ntoks):
                            for ci in range(_ceil_div(ntok, P)):
                                cw = min(P, ntok - ci * P)
                                nc.gpsimd.dma_start(
                                    out_aps[bi][ds(ci * P, cw), lo:hi],
                                    yrows[bi][ci][:cw, lo:hi],
                                )
                lo, hi = drain_final
                for bi, ntok in enumerate(ntoks):
                    for ci in range(_ceil_div(ntok, P)):
                        cw = min(P, ntok - ci * P)
                        eng = nc.sync if ci % 2 == 0 else nc.scalar
                        eng.dma_start(
                            out_aps[bi][ds(ci * P, cw), lo:hi],
                            yrows[bi][ci][:cw, lo:hi],
                        )

            def load_first_block_interleaved(npieces=4):
                """Interleave the first shared block's sw1(s0)/xs(h0) piece
                DMAs across both HWDGE rings so the very first matmul's
                inputs are at the head of the queues."""
                w1_t, x_tiles = [], []
                for h in range(2):
                    w1h = pw1.tile([P, KDH, FD], bf16, tag="w1", name="w1h")
                    xh = pxe.tile([P, KDH, NSH], bf16, tag="xe", name="xh")
                    step = KDH // npieces
                    for i in range(npieces):
                        sl = slice(i * step, (i + 1) * step)
                        nc.sync.dma_start(
                            w1h[:, sl], sw1p[:][0][:, h * KDH :][:, sl]
                        )
                        nc.scalar.dma_start(
                            xh[:, sl], xsp[:][0][:, h * KDH :][:, sl]
                        )
                    w1_t.append(w1h)
                    x_tiles.append(xh)
                return w1_t, x_tiles

            # combine weights, token-chunk-major: [P, NCH] per sub-block
            NCH = _ceil_div(C, P)
            wgt_sb = pb.tile([P, NEB * NCH], f32, tag="wg")

            def routed_scale(eb):
                return lambda ci, cw: wgt_sb[:cw, eb * NCH + ci : eb * NCH + ci + 1]

            # ---------------- shared experts first (token-parallel) --------
            # The shared phase is PE-bound and DMA-light, so running it first
            # lets the DMA-bound routed phase prefetch its inputs behind the
            # shared matmuls.  h-outer / s-inner ordering; one fused mm2
            # streams each shared w2 once for all four (h, s) sub-blocks.
            sw1_t = [None, None]
            sw1_t[0], xs_h0 = load_first_block_interleaved()
            sw1_t[1] = load_khalves(pw1, sw1p[:][1], FD, "w1", nc.scalar,
                                    npieces=2)
            emit_biases()
            xs_tiles = [
                xs_h0,
                load_khalves(pxe, xsp[:][1], NSH, "xe", nc.sync, npieces=2),
            ]
            gblocks = [[None] * S for _ in range(2)]
            for s in range(S):
                for h in range(2):
                    gblocks[h][s] = mm1(xs_tiles[h], NSH, sw1_t[s], sb1_sb, s * KF)
            # shared mm2 is deferred to the very end: its w2 stream (7.9MB)
            # would otherwise collide with the routed input streams mid-kernel,
            # while at the end the PE has ~70us of matmuls and the DMA rings
            # are otherwise idle.
            shared_pending = (
                gblocks,
                [sw2p[:][s] for s in range(S)],
                [NSH, NSH],
                [ysE[:][ds(h * NSH, NSH), :] for h in range(2)],
                [None, None],
            )
            pending = None

            # ---------------- routed experts ----------------
            # software pipeline: mm1(block i+1) is emitted before mm2(block i)
            # so the PE never waits on the gelu tail or w2 prefetch at a
            # block boundary.
            for eb in range(NEB):
                e = eb // NB
                w1_t = load_khalves(pw1, w1p[:][e], FD, "w1", nc.sync)
                x_tiles = load_khalves(pxe, xep[:][eb], C, "xe", nc.sync)
                nc.sync.dma_start(
                    wgt_sb[:, eb * NCH : (eb + 1) * NCH],
                    wgt[:][eb].rearrange("(o p) -> p o", p=P),
                )
                gt = mm1(x_tiles, C, w1_t, rb1_sb, e * KF)
                if pending is not None:
                    mm2(*pending)
                pending = ([[gt]], [w2p[:][e]], [C], [yE[:][eb]],
                           [routed_scale(eb)])
            mm2(*pending)
            mm2(*shared_pending)

    nc.compile()
    return nc


def kernel(x, shared_w1, shared_b1, shared_w2, shared_b2,
           routed_w1, routed_b1, routed_w2, routed_b2, gate_w, gate_b):
    global LAST_EXEC_NS, LAST_MEAN_EXEC_NS, LAST_TRACE

    x = np.asarray(x, np.float32)
    x2d = np.ascontiguousarray(x.reshape(N, D))

    # ---- gating / routing (control plane) ----
    logits = x2d @ np.asarray(gate_w, np.float32) + np.asarray(gate_b, np.float32)
    logits -= logits.max(axis=-1, keepdims=True)
    probs = np.exp(logits)
    probs /= probs.sum(axis=-1, keepdims=True)                  # [N, E]
    top2 = np.argpartition(-probs, K - 1, axis=-1)[:, :K]       # [N, K]
    sel = np.zeros((N, E), np.bool_)
    sel[np.arange(N)[:, None], top2] = True

    idx_per_e = [np.nonzero(sel[:, e])[0] for e in range(E)]
    counts = np.array([len(i) for i in idx_per_e])
    cmax = max(128, int(np.ceil(counts.max() / 32)) * 32)
    NB = _ceil_div(cmax, 512)        # sub-blocks per expert (1 unless skewed)
    C = max(128, int(np.ceil(cmax / NB / 32)) * 32)

    x_bf = x2d.astype(BF16)

    def pmajor_T(rows):
        """[n, D] fp32/bf16 rows -> x^T p-major [P, KD, n]."""
        return rows.T.reshape(KD, P, rows.shape[0]).transpose(1, 0, 2)

    # pre-swizzled shared weights (identical on every core)
    sw1p = np.ascontiguousarray(
        np.asarray(shared_w1).astype(BF16).reshape(S, KD, P, FD).transpose(0, 2, 1, 3)
    )
    sw2p = np.ascontiguousarray(
        np.asarray(shared_w2).astype(BF16)
        .reshape(S, KF, P, NDC // 2, 1024).transpose(0, 3, 2, 1, 4)
    )
    rw1_bf = np.asarray(routed_w1).astype(BF16)
    rw2_bf = np.asarray(routed_w2).astype(BF16)
    rb1_f = np.asarray(routed_b1, np.float32)
    sb1_f = np.asarray(shared_b1, np.float32)

    CP = _ceil_div(C, P) * P
    in_maps = []
    for c in range(NCORES):
        xep = np.zeros((EPC * NB, P, KD, C), BF16)
        wgtb = np.zeros((EPC * NB, CP), np.float32)
        for j in range(EPC):
            e = c * EPC + j
            for b in range(NB):
                idx = idx_per_e[e][b * C : (b + 1) * C]
                if len(idx) == 0:
                    continue
                xep[j * NB + b, :, :, : len(idx)] = pmajor_T(x_bf[idx])
                wgtb[j * NB + b, : len(idx)] = probs[idx, e]
        xsp = np.stack(
            [
                pmajor_T(x_bf[c * NS + h * NSH : c * NS + (h + 1) * NSH])
                for h in range(2)
            ]
        )
        w1c = rw1_bf[c * EPC : (c + 1) * EPC]
        w2c = rw2_bf[c * EPC : (c + 1) * EPC]
        in_maps.append(
            {
                "xep": xep,
                "wgt": wgtb,
                "w1p": np.ascontiguousarray(
                    w1c.reshape(EPC, KD, P, FD).transpose(0, 2, 1, 3)
                ),
                "rb1": np.ascontiguousarray(rb1_f[c * EPC : (c + 1) * EPC]),
                "w2p": np.ascontiguousarray(
                    w2c.reshape(EPC, KF, P, NDC // 2, 1024).transpose(0, 3, 2, 1, 4)
                ),
                "xsp": np.ascontiguousarray(xsp),
                "sw1p": sw1p,
                "sb1": sb1_f,
                "sw2p": sw2p,
            }
        )

    nc = _build_nc(C, NB)
    trace = bool(int(os.environ.get("MOE_TRACE", "0")))
    res = None
    for attempt in range(3):
        try:
            res = run_bass_kernel_spmd(
                nc, in_maps, core_ids=list(range(NCORES)), trace=trace
            )
            break
        except Exception:
            if attempt == 2:
                raise
    LAST_EXEC_NS = res.exec_time_ns
    LAST_MEAN_EXEC_NS = res.mean_exec_time_ns
    LAST_TRACE = res.instructions_and_trace[1] if res.instructions_and_trace else None

    # ---- combine (un-shard) ----
    out = np.zeros((N, D), np.float32)
    for c in range(NCORES):
        out[c * NS : (c + 1) * NS] = np.asarray(res.results[c]["ysE"], np.float32)
    for c in range(NCORES):
        yEc = res.results[c]["yE"]
        for j in range(EPC):
            e = c * EPC + j
            for b in range(NB):
                idx = idx_per_e[e][b * C : (b + 1) * C]
                if len(idx):
                    out[idx] += np.asarray(
                        yEc[j * NB + b, : len(idx), :], np.float32
                    )

    # host-side bias terms (zero in the shipped init; guarded for generality)
    shared_b2 = np.asarray(shared_b2, np.float32)
    if np.any(shared_b2):
        out += shared_b2.sum(0)[None, :]
    routed_b2 = np.asarray(routed_b2, np.float32)
    if np.any(routed_b2):
        w_full = np.where(sel, probs, 0.0).astype(np.float32)
        out += w_full @ routed_b2

    return out.reshape(B, T, D)
